# revision 12
# baseline (speedup 1.0000x reference)
"""int4 weight-only quantized GEMV on 8 TRN2 NeuronCores.

out[1, n] = sum_k A[1, k] * W[n, k],   W = dequant(B packed nibbles, scales/zeros)
A: [1, 8192] fp16, B: [16384, 4096] int32 (one byte per elem, 2 nibbles),
scalesAndZeros: [16384, 256, 2] fp16 (group=32 along K).

Sharding: N=16384 rows split across 8 cores (2048 rows each); A replicated.

Per-core algorithm (tensor-parallel column shard), per 128-row tile:
  lo = B & 15, hi = B >> 4                       (DVE tensor_scalar, int32 -> fp16)
  prod = [lo|hi] * [Ae|Ao]_bcast                 (DVE tensor_tensor, fp16 2x)
  q = prod_lo + prod_hi                          (parity fold)
  dotg[n,g] = segmented sum of q by 16           (strided tree adds, fp16 2x)
  out[n] = sum_g s[n,g]*(dotg[n,g] - 8*sAg[g]) + z[n,g]*sAg[g]
           (two chained tensor_tensor_reduce ops, fp32 accum)
"""

import numpy as np
import sys

import concourse.bass as bass
import concourse.bacc as bacc
import concourse.mybir as mybir
from concourse import tile
from concourse.bass_utils import run_bass_kernel_spmd

FP16 = mybir.dt.float16
FP32 = mybir.dt.float32
INT32 = mybir.dt.int32
Alu = mybir.AluOpType

M, K, N = 1, 8192, 16384
KH = K // 2          # 4096 packed bytes per row
GROUP = 32
NG = K // GROUP      # 256 groups
NCORES = 8
NS = N // NCORES     # 2048 rows per core
P = 128              # partitions per tile
NT = NS // P         # 16 tiles per core


def build_program(ns=NS):
    nt = ns // P
    nc = bacc.Bacc()
    a_d = nc.declare_dram_parameter("A", [M, K], FP16, isOutput=False)
    b_d = nc.declare_dram_parameter("B", [ns, KH], INT32, isOutput=False)
    sz_d = nc.declare_dram_parameter("SZ", [ns, NG, 2], FP16, isOutput=False)
    out_d = nc.declare_dram_parameter("OUT", [ns], FP16, isOutput=True)

    with tile.TileContext(nc) as tc:
        with (
            tc.tile_pool(name="const", bufs=1) as cpool,
            tc.tile_pool(name="bin", bufs=2) as bpool,
            tc.tile_pool(name="work", bufs=2) as wpool,
            tc.tile_pool(name="small", bufs=3) as spool,
        ):
            # --- one-time setup: A broadcast in [Ae | Ao] layout ---
            a_sb = cpool.tile([P, K], FP16)
            a_stage = cpool.tile([1, K], FP16)
            a_v = a_d.rearrange("m (j t) -> m t j", t=2)  # [1, 2, 4096] = [Ae|Ao]
            nc.sync.dma_start(
                out=a_stage.rearrange("m (t j) -> m t j", t=2), in_=a_v
            )
            a_bounce = nc.dram_tensor("a_bounce", [1, K], FP16)
            nc.sync.dma_start(out=a_bounce[:, :], in_=a_stage[0:1, :])
            nc.sync.dma_start(
                out=a_sb[:, :], in_=a_bounce[0:1, :].broadcast_to([P, K])
            )

            # sAg[g] = sum of A over group g (fp32), c8 = 8*sAg (fp16)
            sa2 = cpool.tile([P, 2 * NG], FP32)
            nc.vector.tensor_reduce(
                out=sa2.rearrange("p (t g) -> p t g", t=2),
                in_=a_sb.rearrange("p (t g j) -> p t g j", t=2, g=NG),
                axis=mybir.AxisListType.X,
                op=Alu.add,
            )
            sag = cpool.tile([P, NG], FP32)
            nc.vector.tensor_tensor(
                out=sag[:, :], in0=sa2[:, 0:NG], in1=sa2[:, NG : 2 * NG], op=Alu.add
            )
            c8 = cpool.tile([P, NG], FP16)
            nc.vector.tensor_scalar(
                out=c8[:, :], in0=sag[:, :], scalar1=8.0, scalar2=None, op0=Alu.mult
            )

            for t in range(nt):
                r0 = t * P
                bt = bpool.tile([P, KH], INT32, tag="bt")
                nc.gpsimd.dma_start(out=bt[:, :], in_=b_d[r0 : r0 + P, :])
                szt = bpool.tile([P, 2 * NG], FP16, tag="szt")
                nc.gpsimd.dma_start(
                    out=szt[:, :],
                    in_=sz_d[r0 : r0 + P].rearrange("n g t -> n (g t)"),
                )

                nib = wpool.tile([P, K], INT32, tag="nib")
                nc.vector.tensor_scalar(
                    out=nib[:, 0:KH], in0=bt[:, :], scalar1=15, scalar2=None,
                    op0=Alu.bitwise_and,
                )
                nc.vector.tensor_scalar(
                    out=nib[:, KH:K], in0=bt[:, :], scalar1=4, scalar2=None,
                    op0=Alu.logical_shift_right,
                )

                prod = wpool.tile([P, K], FP16, tag="prod")
                nc.vector.tensor_tensor(
                    out=prod[:, :], in0=nib[:, :], in1=a_sb[:, :], op=Alu.mult
                )

                q = wpool.tile([P, KH], FP16, tag="q")
                nc.vector.tensor_tensor(
                    out=q[:, :], in0=prod[:, 0:KH], in1=prod[:, KH:K], op=Alu.add
                )

                # tree-reduce groups of 16 -> dotg [P, NG]
                r1 = wpool.tile([P, 2048], FP16, tag="r1")
                qv = q.rearrange("p (g j) -> p g j", j=16)
                nc.vector.tensor_tensor(
                    out=r1.rearrange("p (g j) -> p g j", j=8),
                    in0=qv[:, :, 0:8], in1=qv[:, :, 8:16], op=Alu.add,
                )
                r2 = wpool.tile([P, 1024], FP16, tag="r2")
                r1v = r1.rearrange("p (g j) -> p g j", j=8)
                nc.vector.tensor_tensor(
                    out=r2.rearrange("p (g j) -> p g j", j=4),
                    in0=r1v[:, :, 0:4], in1=r1v[:, :, 4:8], op=Alu.add,
                )
                r3 = wpool.tile([P, 512], FP16, tag="r3")
                r2v = r2.rearrange("p (g j) -> p g j", j=4)
                nc.vector.tensor_tensor(
                    out=r3.rearrange("p (g j) -> p g j", j=2),
                    in0=r2v[:, :, 0:2], in1=r2v[:, :, 2:4], op=Alu.add,
                )
                dotg = spool.tile([P, NG], FP16, tag="dotg")
                r3v = r3.rearrange("p (g j) -> p g j", j=2)
                nc.vector.tensor_tensor(
                    out=dotg[:, :], in0=r3v[:, :, 0], in1=r3v[:, :, 1], op=Alu.add
                )

                # dadj = dotg - 8*sAg
                dadj = spool.tile([P, NG], FP16, tag="dadj")
                nc.vector.tensor_tensor(
                    out=dadj[:, :], in0=dotg[:, :], in1=c8[:, :], op=Alu.subtract
                )

                szv = szt.rearrange("p (g t) -> p g t", t=2)
                scr1 = spool.tile([P, NG], FP16, tag="scr1")
                nc.vector.tensor_tensor(
                    out=scr1[:, :], in0=dadj[:, :], in1=szv[:, :, 0], op=Alu.mult
                )
                acc1 = spool.tile([P, 1], FP32, tag="acc1")
                nc.vector.tensor_reduce(
                    out=acc1[:, :], in_=scr1[:, :], axis=mybir.AxisListType.X,
                    op=Alu.add,
                )
                scr2 = spool.tile([P, NG], FP32, tag="scr2")
                nc.vector.tensor_tensor(
                    out=scr2[:, :], in0=szv[:, :, 1], in1=sag[:, :], op=Alu.mult
                )
                acc2 = spool.tile([P, 1], FP32, tag="acc2")
                nc.vector.tensor_reduce(
                    out=acc2[:, :], in_=scr2[:, :], axis=mybir.AxisListType.X,
                    op=Alu.add,
                )

                outt = spool.tile([P, 1], FP16, tag="outt")
                nc.vector.tensor_tensor(
                    out=outt[:, :], in0=acc1[:, :], in1=acc2[:, :], op=Alu.add
                )
                nc.gpsimd.dma_start(out=out_d[r0 : r0 + P], in_=outt[:, 0:1])
    nc.finalize()
    return nc


_NC_CACHE = {}


def _get_program(ns=NS):
    if ns not in _NC_CACHE:
        _NC_CACHE[ns] = build_program(ns)
    return _NC_CACHE[ns]


def kernel(A, B, scalesAndZeros):
    A = np.asarray(A)
    B = np.asarray(B)
    SZ = np.asarray(scalesAndZeros)
    nc = _get_program()
    in_maps = []
    for c in range(NCORES):
        r0, r1 = c * NS, (c + 1) * NS
        in_maps.append(
            {
                "A": np.ascontiguousarray(A),
                "B": np.ascontiguousarray(B[r0:r1]),
                "SZ": np.ascontiguousarray(SZ[r0:r1]),
            }
        )
    res = run_bass_kernel_spmd(nc, in_maps, core_ids=list(range(NCORES)))
    out = np.concatenate([res.results[c]["OUT"] for c in range(NCORES)])
    return out.reshape(1, N).astype(np.float16)


if __name__ == "__main__":
    rng = np.random.default_rng(0)
    A = rng.standard_normal((M, K)).astype(np.float16)
    B = rng.integers(0, 256, (N, KH)).astype(np.int32)
    SZ = rng.standard_normal((N, NG, 2)).astype(np.float16)
    out = kernel(A, B, SZ)
    print(out.shape, out.dtype, out[0, :8])


# revision 13
# speedup vs baseline: 1.6039x; 1.6039x over previous
"""int4 weight-only quantized GEMV on 8 TRN2 NeuronCores.

out[1, n] = sum_k A[1, k] * W[n, k],   W = dequant(B packed nibbles, scales/zeros)
A: [1, 8192] fp16, B: [16384, 4096] int32 (one byte per elem, 2 nibbles),
scalesAndZeros: [16384, 256, 2] fp16 (group=32 along K).

Sharding: N=16384 rows split across 8 cores (2048 rows each); A replicated.

Host-side prep (cheap, O(K)): A is deinterleaved into [Ae | Ao] halves and
replicated across the 128 SBUF partitions; per-group sums of A are
precomputed ([128, 256] fp32). Device does all O(N*K) work.

Per-core, per 128-row tile:
  bt   = B tile, DMA-cast int32 -> int16 on load       (SWDGE cast)
  lo   = bt & 15, hi = bt >> 4                         (DVE tensor_scalar int16)
  prod = [lo|hi] * [Ae|Ao]_bcast                       (DVE tensor_tensor fp16 2x)
  q    = prod_lo + prod_hi                             (parity fold)
  dotg = tree-sum of q in groups of 16                 (strided TT adds, 2x)
  out[n] = sum_g s[n,g]*(dotg[n,g] - 8*sAg[g]) + z[n,g]*sAg[g]
"""

import numpy as np

import concourse.bass as bass
import concourse.bacc as bacc
import concourse.mybir as mybir
from concourse import tile
from concourse.bass_utils import run_bass_kernel_spmd

FP16 = mybir.dt.float16
FP32 = mybir.dt.float32
INT16 = mybir.dt.int16
INT32 = mybir.dt.int32
Alu = mybir.AluOpType

M, K, N = 1, 8192, 16384
KH = K // 2          # 4096 packed bytes per row
GROUP = 32
NG = K // GROUP      # 256 groups
NCORES = 8
NS = N // NCORES     # 2048 rows per core
P = 128              # partitions per tile
NT = NS // P         # 16 tiles per core


def build_program(ns=NS):
    nt = ns // P
    nc = bacc.Bacc()
    ab_d = nc.declare_dram_parameter("AB", [P, K], FP16, isOutput=False)
    sag_d = nc.declare_dram_parameter("SAG", [P, NG], FP32, isOutput=False)
    b_d = nc.declare_dram_parameter("B", [ns, KH], INT32, isOutput=False)
    sz_d = nc.declare_dram_parameter("SZ", [ns, NG, 2], FP16, isOutput=False)
    out_d = nc.declare_dram_parameter("OUT", [ns], FP16, isOutput=True)

    with tile.TileContext(nc) as tc:
        with (
            tc.tile_pool(name="const", bufs=1) as cpool,
            tc.tile_pool(name="bin", bufs=3) as bpool,
            tc.tile_pool(name="work", bufs=2) as wpool,
            tc.tile_pool(name="small", bufs=3) as spool,
        ):
            a_sb = cpool.tile([P, K], FP16)
            nc.sync.dma_start(out=a_sb[:, :], in_=ab_d[:, :])
            sag = cpool.tile([P, NG], FP32)
            nc.sync.dma_start(out=sag[:, :], in_=sag_d[:, :])
            c8 = cpool.tile([P, NG], FP16)
            nc.vector.tensor_scalar(
                out=c8[:, :], in0=sag[:, :], scalar1=8.0, scalar2=None, op0=Alu.mult
            )

            for t in range(nt):
                r0 = t * P
                bt = bpool.tile([P, KH], INT16, tag="bt")
                nc.gpsimd.dma_start(out=bt[:, :], in_=b_d[r0 : r0 + P, :])
                szt = bpool.tile([P, 2 * NG], FP16, tag="szt")
                nc.gpsimd.dma_start(
                    out=szt[:, :],
                    in_=sz_d[r0 : r0 + P].rearrange("n g t -> n (g t)"),
                )

                nib = wpool.tile([P, K], INT16, tag="nib")
                nc.vector.tensor_scalar(
                    out=nib[:, 0:KH], in0=bt[:, :], scalar1=15, scalar2=None,
                    op0=Alu.bitwise_and,
                )
                nc.vector.tensor_scalar(
                    out=nib[:, KH:K], in0=bt[:, :], scalar1=4, scalar2=None,
                    op0=Alu.logical_shift_right,
                )

                prod = wpool.tile([P, K], FP16, tag="prod")
                nc.vector.tensor_tensor(
                    out=prod[:, :], in0=nib[:, :], in1=a_sb[:, :], op=Alu.mult
                )

                q = wpool.tile([P, KH], FP16, tag="q")
                nc.vector.tensor_tensor(
                    out=q[:, :], in0=prod[:, 0:KH], in1=prod[:, KH:K], op=Alu.add
                )

                # tree-reduce groups of 16 -> dotg [P, NG]
                r1 = wpool.tile([P, 2048], FP16, tag="r1")
                qv = q.rearrange("p (g j) -> p g j", j=16)
                nc.vector.tensor_tensor(
                    out=r1.rearrange("p (g j) -> p g j", j=8),
                    in0=qv[:, :, 0:8], in1=qv[:, :, 8:16], op=Alu.add,
                )
                r2 = wpool.tile([P, 1024], FP16, tag="r2")
                r1v = r1.rearrange("p (g j) -> p g j", j=8)
                nc.vector.tensor_tensor(
                    out=r2.rearrange("p (g j) -> p g j", j=4),
                    in0=r1v[:, :, 0:4], in1=r1v[:, :, 4:8], op=Alu.add,
                )
                r3 = wpool.tile([P, 512], FP16, tag="r3")
                r2v = r2.rearrange("p (g j) -> p g j", j=4)
                nc.vector.tensor_tensor(
                    out=r3.rearrange("p (g j) -> p g j", j=2),
                    in0=r2v[:, :, 0:2], in1=r2v[:, :, 2:4], op=Alu.add,
                )
                dotg = spool.tile([P, NG], FP16, tag="dotg")
                r3v = r3.rearrange("p (g j) -> p g j", j=2)
                nc.vector.tensor_tensor(
                    out=dotg[:, :], in0=r3v[:, :, 0], in1=r3v[:, :, 1], op=Alu.add
                )

                dadj = spool.tile([P, NG], FP16, tag="dadj")
                nc.vector.tensor_tensor(
                    out=dadj[:, :], in0=dotg[:, :], in1=c8[:, :], op=Alu.subtract
                )

                szv = szt.rearrange("p (g t) -> p g t", t=2)
                scr1 = spool.tile([P, NG], FP16, tag="scr1")
                nc.vector.tensor_tensor(
                    out=scr1[:, :], in0=dadj[:, :], in1=szv[:, :, 0], op=Alu.mult
                )
                acc1 = spool.tile([P, 1], FP32, tag="acc1")
                nc.vector.tensor_reduce(
                    out=acc1[:, :], in_=scr1[:, :], axis=mybir.AxisListType.X,
                    op=Alu.add,
                )
                scr2 = spool.tile([P, NG], FP32, tag="scr2")
                nc.vector.tensor_tensor(
                    out=scr2[:, :], in0=szv[:, :, 1], in1=sag[:, :], op=Alu.mult
                )
                acc2 = spool.tile([P, 1], FP32, tag="acc2")
                nc.vector.tensor_reduce(
                    out=acc2[:, :], in_=scr2[:, :], axis=mybir.AxisListType.X,
                    op=Alu.add,
                )

                outt = spool.tile([P, 1], FP16, tag="outt")
                nc.vector.tensor_tensor(
                    out=outt[:, :], in0=acc1[:, :], in1=acc2[:, :], op=Alu.add
                )
                nc.gpsimd.dma_start(out=out_d[r0 : r0 + P], in_=outt[:, 0:1])
    nc.finalize()
    return nc


_NC_CACHE = {}


def _get_program(ns=NS):
    if ns not in _NC_CACHE:
        _NC_CACHE[ns] = build_program(ns)
    return _NC_CACHE[ns]


def _prep_a(A):
    """Deinterleave A into [Ae | Ao], replicate to 128 partitions; per-group
    sums of A broadcast likewise."""
    a = np.asarray(A).reshape(K)
    a_de = np.concatenate([a[0::2], a[1::2]]).astype(np.float16)
    ab = np.ascontiguousarray(np.broadcast_to(a_de, (P, K)))
    sag = a.astype(np.float32).reshape(NG, GROUP).sum(-1)
    sag_b = np.ascontiguousarray(np.broadcast_to(sag, (P, NG)))
    return ab, sag_b


def kernel(A, B, scalesAndZeros):
    A = np.asarray(A)
    B = np.asarray(B)
    SZ = np.asarray(scalesAndZeros)
    ab, sag_b = _prep_a(A)
    nc = _get_program()
    in_maps = []
    for c in range(NCORES):
        r0, r1 = c * NS, (c + 1) * NS
        in_maps.append(
            {
                "AB": ab,
                "SAG": sag_b,
                "B": np.ascontiguousarray(B[r0:r1]),
                "SZ": np.ascontiguousarray(SZ[r0:r1]),
            }
        )
    res = run_bass_kernel_spmd(nc, in_maps, core_ids=list(range(NCORES)))
    out = np.concatenate([res.results[c]["OUT"] for c in range(NCORES)])
    return out.reshape(1, N).astype(np.float16)


if __name__ == "__main__":
    rng = np.random.default_rng(0)
    A = rng.standard_normal((M, K)).astype(np.float16)
    B = rng.integers(0, 256, (N, KH)).astype(np.int32)
    SZ = rng.standard_normal((N, NG, 2)).astype(np.float16)
    out = kernel(A, B, SZ)
    print(out.shape, out.dtype, out[0, :8])
